# revision 1
# baseline (speedup 1.0000x reference)
"""CrossAttentionFusion Trainium2 kernel (nn_CrossAttentionFusion__45561013076033).

Full inputs -> full output. Sharding: 8 cores, core c handles batch b=c//2,
query-half h=c%2 (2048 of 4096 queries). Each core holds the full [256,4096]
cnn feature map of its batch (keys/values), its query-half of the transformer
features, and replicated weights.

Per-core dataflow (channel-major [C, N] layouts throughout):
  Q = (Wq X_trf + bq)/16          [256, 2048]  fp32r matmuls
  K = Wk X_cnn + bk               [256, 4096]
  V^T = X_cnn^T Wv^T              [4096, 256]  (bv folded into conv bias)
  per 128-query block:
    S = Q_blk^T K                 [128, 4096]  fp32r
    P = exp(S) (no max-sub; scores are O(1)), row sums via ACT accum_out
    PT = P^T diag(256/rowsum)     via fp16 matmul with scaled identity
  per 256-query superblock:
    A' = V^T^T PT = 256 * attended_norm   [256, 256]  fp16 matmuls, fp32 acc
    out = Wf1 X_trf + (Wf2/256) A' + (bf + Wf2 bv)    fp32r
"""

import numpy as np

B, C, H, W = 4, 256, 64, 64
N = H * W            # 4096 tokens
NCORES = 8
QH = N // 2          # 2048 queries per core
CT = C // 128        # 2 channel tiles
KC = N // 512        # 8 key chunks of 512
NQB = QH // 128      # 16 query blocks per core
NSB = QH // 512      # 4 superblocks per core
NKT = N // 128       # 32 key tiles

_CACHE = {}


def _build(bench_reps=None, dma_outside=False):
    import contextlib
    import concourse.bass as bass
    import concourse.mybir as mybir
    import concourse.tile as tile
    from concourse import bacc
    from concourse.masks import make_identity

    f32 = mybir.dt.float32
    f32r = mybir.dt.float32r
    f16 = mybir.dt.float16
    AF = mybir.ActivationFunctionType

    nc = bacc.Bacc("TRN2", target_bir_lowering=False, debug=True)

    XQ = nc.dram_tensor("xq", [C, QH], f32, kind="ExternalInput")
    XC = nc.dram_tensor("xc", [C, N], f32, kind="ExternalInput")
    WQT = nc.dram_tensor("wqt", [C, C], f32, kind="ExternalInput")
    WKT = nc.dram_tensor("wkt", [C, C], f32, kind="ExternalInput")
    WVT = nc.dram_tensor("wvt", [C, C], f32, kind="ExternalInput")
    WFT = nc.dram_tensor("wft", [2 * C, C], f32, kind="ExternalInput")
    BQ = nc.dram_tensor("bq", [C], f32, kind="ExternalInput")
    BK = nc.dram_tensor("bk", [C], f32, kind="ExternalInput")
    BF = nc.dram_tensor("bf", [C], f32, kind="ExternalInput")
    OUT = nc.dram_tensor("out", [C, QH], f32, kind="ExternalOutput")

    xq_d = XQ.ap().bitcast(f32r).rearrange("(t p) n -> p t n", p=128)
    xc_d = XC.ap().bitcast(f32r).rearrange("(t p) n -> p t n", p=128)
    wq_d = WQT.ap().bitcast(f32r).rearrange("(t p) d -> p t d", p=128)
    wk_d = WKT.ap().bitcast(f32r).rearrange("(t p) d -> p t d", p=128)
    wv_d = WVT.ap().bitcast(f32r).rearrange("(t p) d -> p t d", p=128)
    wf_d = WFT.ap().bitcast(f32r).rearrange("(t p) d -> p t d", p=128)
    out_d = OUT.ap().rearrange("(t p) n -> p t n", p=128)

    with tile.TileContext(nc) as tc:
        with tc.tile_pool(name="persist", bufs=1) as per, \
             tc.tile_pool(name="soft", bufs=2) as soft, \
             tc.tile_pool(name="ptp", bufs=1) as ptp, \
             tc.tile_pool(name="outp", bufs=2) as outp, \
             tc.tile_pool(name="mm", bufs=2, space="PSUM") as mmp, \
             tc.tile_pool(name="tp", bufs=2, space="PSUM") as tpp, \
             tc.tile_pool(name="av", bufs=2, space="PSUM") as avp:

            # ---- persistent tiles ----
            xq_sb = per.tile([128, CT, QH], f32r)
            xc_sb = per.tile([128, CT, N], f32r)
            wq_sb = per.tile([128, CT, C], f32r)
            wk_sb = per.tile([128, CT, C], f32r)
            wv_sb = per.tile([128, CT, C], f32r)
            wf_sb = per.tile([128, 2 * CT, C], f32r)
            bq_sb = per.tile([128, CT], f32)
            bk_sb = per.tile([128, CT], f32)
            bf_sb = per.tile([128, CT], f32)
            q_sb = per.tile([128, CT, QH], f32r)
            k_sb = per.tile([128, CT, N], f32r)
            vt_sb = per.tile([128, NKT, C], f16)
            ident = per.tile([128, 128], f16)

            nc.sync.dma_start(wq_sb[:], wq_d)
            nc.sync.dma_start(wk_sb[:], wk_d)
            nc.sync.dma_start(wv_sb[:], wv_d)
            nc.sync.dma_start(wf_sb[:], wf_d)
            nc.sync.dma_start(bq_sb[:], BQ.ap().rearrange("(t p) -> p t", p=128))
            nc.sync.dma_start(bk_sb[:], BK.ap().rearrange("(t p) -> p t", p=128))
            nc.sync.dma_start(bf_sb[:], BF.ap().rearrange("(t p) -> p t", p=128))
            make_identity(nc, ident[:])

            env = dict(locals())
            if dma_outside:
                _emit_input_dmas(nc, env)
            loop_cm = (tc.For_i(0, bench_reps, 1) if bench_reps
                       else contextlib.nullcontext())
            with loop_cm:
                _emit_body(nc, tc, mybir, env, skip_dmas=dma_outside)
    nc.finalize()
    return nc


DMA_CHUNKED = True


def _emit_input_dmas(nc, env):
    xq_sb, xc_sb = env["xq_sb"], env["xc_sb"]
    xq_d, xc_d = env["xq_d"], env["xc_d"]
    if not DMA_CHUNKED:
        for ct in range(CT):
            nc.sync.dma_start(xq_sb[:, ct], xq_d[:, ct])
        for ct in range(CT):
            nc.sync.dma_start(xc_sb[:, ct], xc_d[:, ct])
        return
    for qc in range(QH // 512):
        for ct in range(CT):
            s = slice(qc * 512, (qc + 1) * 512)
            nc.sync.dma_start(xq_sb[:, ct, s], xq_d[:, ct, s])
    for kc in range(KC):
        for ct in range(CT):
            s = slice(kc * 512, (kc + 1) * 512)
            nc.sync.dma_start(xc_sb[:, ct, s], xc_d[:, ct, s])


def _emit_body(nc, tc, mybir, env, skip_dmas=False):
    f32 = mybir.dt.float32
    f32r = mybir.dt.float32r
    f16 = mybir.dt.float16
    AF = mybir.ActivationFunctionType
    (xq_sb, xc_sb, wq_sb, wk_sb, wv_sb, wf_sb, bq_sb, bk_sb, bf_sb,
     q_sb, k_sb, vt_sb, ident) = (
        env["xq_sb"], env["xc_sb"], env["wq_sb"], env["wk_sb"], env["wv_sb"],
        env["wf_sb"], env["bq_sb"], env["bk_sb"], env["bf_sb"],
        env["q_sb"], env["k_sb"], env["vt_sb"], env["ident"])
    soft, ptp, outp, mmp, tpp, avp = (
        env["soft"], env["ptp"], env["outp"], env["mmp"], env["tpp"], env["avp"])
    xq_d, xc_d, out_d = env["xq_d"], env["xc_d"], env["out_d"]

    if not skip_dmas:
        _emit_input_dmas(nc, env)

    if True:
            # ---- Q projection: Q[d, n] (scaled by 1/16 via host weights) ----
            for dt in range(CT):
                for qc in range(QH // 512):
                    ps = mmp.tile([128, 512], f32, tag="mm512")
                    for ct in range(CT):
                        nc.tensor.matmul(
                            ps[:], wq_sb[:, ct, dt * 128:(dt + 1) * 128],
                            xq_sb[:, ct, qc * 512:(qc + 1) * 512],
                            start=(ct == 0), stop=(ct == CT - 1))
                    nc.scalar.activation(
                        q_sb[:, dt, qc * 512:(qc + 1) * 512], ps[:],
                        AF.Identity, bias=bq_sb[:, dt:dt + 1])

            # ---- K projection ----
            for dt in range(CT):
                for kc in range(KC):
                    ps = mmp.tile([128, 512], f32, tag="mm512")
                    for ct in range(CT):
                        nc.tensor.matmul(
                            ps[:], wk_sb[:, ct, dt * 128:(dt + 1) * 128],
                            xc_sb[:, ct, kc * 512:(kc + 1) * 512],
                            start=(ct == 0), stop=(ct == CT - 1))
                    nc.scalar.activation(
                        k_sb[:, dt, kc * 512:(kc + 1) * 512], ps[:],
                        AF.Identity, bias=bk_sb[:, dt:dt + 1])

            # ---- V^T: [keys, d] (no bias; folded into conv bias) ----
            for mt in range(NKT):
                ps = mmp.tile([128, 512], f32, tag="mm512")
                for ct in range(CT):
                    nc.tensor.matmul(
                        ps[:, :C], xc_sb[:, ct, mt * 128:(mt + 1) * 128],
                        wv_sb[:, ct],
                        start=(ct == 0), stop=(ct == CT - 1))
                nc.scalar.activation(vt_sb[:, mt], ps[:, :C], AF.Copy)

            # ---- attention + fused conv, per 512-query superblock ----
            for sb in range(NSB):
                pt_sb = ptp.tile([128, NKT, 512], f16, tag="pt")
                for qj in range(4):
                    qb = 4 * sb + qj
                    p_sb = soft.tile([128, N], f16, tag="p")
                    sums = soft.tile([128, KC], f32, tag="sums")
                    # S = Q_blk^T K, chunk by 512 keys; exp + row-sum
                    for kc in range(KC):
                        ps = mmp.tile([128, 512], f32, tag="mm512")
                        for ct in range(CT):
                            nc.tensor.matmul(
                                ps[:], q_sb[:, ct, qb * 128:(qb + 1) * 128],
                                k_sb[:, ct, kc * 512:(kc + 1) * 512],
                                start=(ct == 0), stop=(ct == CT - 1))
                        nc.scalar.activation(
                            p_sb[:, kc * 512:(kc + 1) * 512], ps[:],
                            AF.Exp, accum_out=sums[:, kc:kc + 1])
                    ssum = soft.tile([128, 1], f32, tag="ssum")
                    nc.vector.reduce_sum(ssum[:], sums[:],
                                         axis=mybir.AxisListType.X)
                    rinv = soft.tile([128, 1], f32, tag="rinv")
                    nc.vector.reciprocal(rinv[:], ssum[:])
                    r256 = soft.tile([128, 1], f32, tag="r256")
                    nc.vector.tensor_scalar_mul(r256[:], rinv[:], 256.0)
                    sid = soft.tile([128, 128], f16, tag="sid")
                    nc.vector.tensor_scalar_mul(sid[:], ident[:], r256[:])
                    # PT[k, q] = P[q, k] * 256/rowsum[q] via fp16 matmul
                    for g in range(NKT // 4):
                        tps = tpp.tile([128, 4, 128], f32, tag="tp")
                        for j in range(4):
                            kt = 4 * g + j
                            nc.tensor.matmul(
                                tps[:, j], p_sb[:, kt * 128:(kt + 1) * 128],
                                sid[:], start=True, stop=True)
                        nc.vector.tensor_copy(
                            pt_sb[:, 4 * g:4 * (g + 1),
                                  qj * 128:(qj + 1) * 128], tps[:])

                # A' = sum_k VT[k, :]^T PT[k, :]  -> [256 d, 512 q]
                aps = avp.tile([128, CT, 512], f32, tag="av")
                for kt in range(NKT):
                    for dt in range(CT):
                        nc.tensor.matmul(
                            aps[:, dt], vt_sb[:, kt, dt * 128:(dt + 1) * 128],
                            pt_sb[:, kt],
                            start=(kt == 0), stop=(kt == NKT - 1))
                a_sb = outp.tile([128, CT, 512], f32r, tag="a")
                nc.scalar.activation(a_sb[:], aps[:], AF.Copy)

                # fused conv: out = Wf1 xq + Wf2' A' + bf2
                o_sb = outp.tile([128, CT, 512], f32, tag="o")
                for dt in range(CT):
                    ops = mmp.tile([128, 512], f32, tag="mm512")
                    for kt in range(2 * CT):
                        rhs = (xq_sb[:, kt, sb * 512:(sb + 1) * 512] if kt < CT
                               else a_sb[:, kt - CT])
                        nc.tensor.matmul(
                            ops[:], wf_sb[:, kt, dt * 128:(dt + 1) * 128],
                            rhs, start=(kt == 0), stop=(kt == 2 * CT - 1))
                    nc.scalar.activation(o_sb[:, dt], ops[:],
                                         AF.Identity, bias=bf_sb[:, dt:dt + 1])
                nc.sync.dma_start(out_d[:, :, sb * 512:(sb + 1) * 512], o_sb[:])


def _get_nc(bench_reps=None, dma_outside=False):
    key = ("nc", bench_reps, dma_outside)
    if key not in _CACHE:
        _CACHE[key] = _build(bench_reps, dma_outside)
    return _CACHE[key]


def _in_maps(transformer_features, cnn_features, Wq, bq, Wk, bk, Wv, bv, Wf, bf):
    xt = np.ascontiguousarray(np.asarray(transformer_features, np.float32)
                              .reshape(B, C, N))
    xc = np.ascontiguousarray(np.asarray(cnn_features, np.float32)
                              .reshape(B, C, N))
    Wq = np.asarray(Wq, np.float32)
    Wk = np.asarray(Wk, np.float32)
    Wv = np.asarray(Wv, np.float32)
    Wf = np.asarray(Wf, np.float32)
    bq = np.asarray(bq, np.float32)
    bk = np.asarray(bk, np.float32)
    bv = np.asarray(bv, np.float32)
    bf = np.asarray(bf, np.float32)

    wqt = np.ascontiguousarray(Wq.T / 16.0)
    wkt = np.ascontiguousarray(Wk.T)
    wvt = np.ascontiguousarray(Wv.T)
    wft = np.ascontiguousarray(Wf.T).copy()
    wft[C:] /= 256.0
    bq_s = bq / 16.0
    bf2 = bf + Wf[:, C:] @ bv

    maps = []
    for c in range(NCORES):
        b, h = divmod(c, 2)
        maps.append(dict(
            xq=np.ascontiguousarray(xt[b][:, h * QH:(h + 1) * QH]),
            xc=xc[b],
            wqt=wqt, wkt=wkt, wvt=wvt, wft=wft,
            bq=bq_s, bk=bk, bf=bf2,
        ))
    return maps


def _run(inputs, trace=False):
    from concourse.bass_utils import run_bass_kernel_spmd
    nc = _get_nc()
    maps = _in_maps(**inputs)
    return run_bass_kernel_spmd(nc, maps, list(range(NCORES)), trace=trace)


def kernel(**inputs) -> np.ndarray:
    res = _run(inputs).results
    out = np.empty((B, C, N), np.float32)
    for c in range(NCORES):
        b, h = divmod(c, 2)
        out[b][:, h * QH:(h + 1) * QH] = res[c]["out"]
    return out.reshape(B, C, H, W)



# revision 3
# speedup vs baseline: 1.3494x; 1.3494x over previous
"""CrossAttentionFusion Trainium2 kernel (nn_CrossAttentionFusion__45561013076033).

Full inputs -> full output. Sharding: 8 cores, core c handles batch b=c//2,
query-half h=c%2 (2048 of 4096 queries). Each core holds the full [256,4096]
cnn feature map of its batch (keys), its query-half of the transformer
features, and replicated weights.

Key algebraic restructuring vs the naive dataflow:
  * The 1x1-conv output splits as out = Wf1 @ x_trf + Wf2 @ attended + bf.
    Fold Wf2 into the value projection: U = (Wf2 @ Wv) @ x_cnn, so the
    attention output directly produces conv-ready channels and the separate
    attended->conv matmul disappears.  bv's contribution is a constant bias
    (softmax rows sum to 1): bf' = bf + Wf2 @ bv.
  * Scores are computed pre-transposed: S^T[k, q] = (K-chunk)^T Q, so the
    P^T needed by the PV matmul comes straight out of exp() -- no transpose
    matmuls over the [N, N] attention matrix.
  * Softmax row-sums come free as a 257th column of the PV matmul by
    augmenting U^T with a ones column: [A | R] = P^T^T [U | 1].
  * Normalization (1/R per query) is applied by the vector engine while
    moving the PV result PSUM->SBUF (per-partition scalar multiply, since
    queries sit on partitions there).
  * The [q, e] -> [e, q] layout fix-up is a matmul with a 128x128 identity
    as the moving operand, accumulated directly into the Wf1 PSUM group.

Per-core dataflow (f32r matmuls except PV/transpose which are fp16):
  Q = (Wq/16) x_trf + bq/16          [256, 2048]
  K = Wk x_cnn + bk                  [256, 4096]
  U^T = x_cnn^T (Wf2 Wv)^T, ones col [4096, 257]   fp16
  per 512-query superblock, per 128-key tile:
    S^T = K_kt^T Q_sb                [128, 512]
    P^T = exp(S^T)                   fp16
  per 128-query block:
    [A | R] = sum_kt P^T_kt^T U'_kt  [128, 257]  fp16 matmuls, fp32 acc
    c = A * (1/R)  (DVE, per-partition scale)     fp16 [128 q, 256 e]
  per (superblock, e-chunk):
    psO = Wf1_et x_trf_sb + sum_qj c_qj^T (identity-matmul accumulate)
    out = psO + bf'                  ACT bias, DMA out
"""

import numpy as np

B, C, H, W = 4, 256, 64, 64
N = H * W            # 4096 tokens
NCORES = 8
QH = N // 2          # 2048 queries per core
CT = C // 128        # 2 channel tiles
KC = N // 512        # 8 key chunks of 512
NSB = QH // 512      # 4 superblocks per core
NKT = N // 128       # 32 key tiles

_CACHE = {}


def _build():
    import concourse.bass as bass
    import concourse.mybir as mybir
    import concourse.tile as tile
    from concourse import bacc
    from concourse.masks import make_identity

    f32 = mybir.dt.float32
    f32r = mybir.dt.float32r
    f16 = mybir.dt.float16
    AF = mybir.ActivationFunctionType

    nc = bacc.Bacc("TRN2", target_bir_lowering=False, debug=True)

    XQ = nc.dram_tensor("xq", [C, QH], f32, kind="ExternalInput")
    XC = nc.dram_tensor("xc", [C, N], f32, kind="ExternalInput")
    WQT = nc.dram_tensor("wqt", [C, C], f32, kind="ExternalInput")
    WKT = nc.dram_tensor("wkt", [C, C], f32, kind="ExternalInput")
    WUT = nc.dram_tensor("wut", [C, C], f32, kind="ExternalInput")
    WF1 = nc.dram_tensor("wf1", [C, C], f32, kind="ExternalInput")
    BQ = nc.dram_tensor("bq", [C], f32, kind="ExternalInput")
    BK = nc.dram_tensor("bk", [C], f32, kind="ExternalInput")
    BF = nc.dram_tensor("bf", [C], f32, kind="ExternalInput")
    OUT = nc.dram_tensor("out", [C, QH], f32, kind="ExternalOutput")

    xq_d = XQ.ap().bitcast(f32r).rearrange("(t p) n -> p t n", p=128)
    xc_d = XC.ap().bitcast(f32r).rearrange("(t p) n -> p t n", p=128)
    wq_d = WQT.ap().bitcast(f32r).rearrange("(t p) d -> p t d", p=128)
    wk_d = WKT.ap().bitcast(f32r).rearrange("(t p) d -> p t d", p=128)
    wu_d = WUT.ap().bitcast(f32r).rearrange("(t p) d -> p t d", p=128)
    wf_d = WF1.ap().bitcast(f32r).rearrange("(t p) d -> p t d", p=128)
    out_d = OUT.ap().rearrange("(t p) n -> p t n", p=128)

    with tile.TileContext(nc) as tc:
        with tc.tile_pool(name="persist", bufs=1) as per, \
             tc.tile_pool(name="pt", bufs=2) as ptp, \
             tc.tile_pool(name="cb", bufs=4) as cbp, \
             tc.tile_pool(name="outp", bufs=2) as outp, \
             tc.tile_pool(name="mm", bufs=2, space="PSUM") as mmp, \
             tc.tile_pool(name="pv", bufs=2, space="PSUM") as pvp, \
             tc.tile_pool(name="po", bufs=2, space="PSUM") as pop:

            # ---- persistent tiles ----
            xq_sb = per.tile([128, CT, QH], f32r)
            xc_sb = per.tile([128, CT, N], f32r)
            wq_sb = per.tile([128, CT, C], f32r)
            wk_sb = per.tile([128, CT, C], f32r)
            wu_sb = per.tile([128, CT, C], f32r)
            wf_sb = per.tile([128, CT, C], f32r)
            bq_sb = per.tile([128, CT], f32)
            bk_sb = per.tile([128, CT], f32)
            bf_sb = per.tile([128, CT], f32)
            q_sb = per.tile([128, CT, QH], f32r)
            k_sb = per.tile([128, CT, N], f32r)
            ut_sb = per.tile([128, NKT, C + 1], f16)
            ident = per.tile([128, 128], f16)

            nc.sync.dma_start(wq_sb[:], wq_d)
            nc.sync.dma_start(wk_sb[:], wk_d)
            nc.sync.dma_start(wu_sb[:], wu_d)
            nc.sync.dma_start(wf_sb[:], wf_d)
            nc.sync.dma_start(bq_sb[:], BQ.ap().rearrange("(t p) -> p t", p=128))
            nc.sync.dma_start(bk_sb[:], BK.ap().rearrange("(t p) -> p t", p=128))
            nc.sync.dma_start(bf_sb[:], BF.ap().rearrange("(t p) -> p t", p=128))
            make_identity(nc, ident[:])
            nc.gpsimd.memset(ut_sb[:, :, C:C + 1], 1.0)

            # input feature DMAs, 512-column chunks, xq/xc interleaved so
            # both Q and K projections can start early
            for i in range(KC):
                if i < QH // 512:
                    for ct in range(CT):
                        s = slice(i * 512, (i + 1) * 512)
                        nc.sync.dma_start(xq_sb[:, ct, s], xq_d[:, ct, s])
                for ct in range(CT):
                    s = slice(i * 512, (i + 1) * 512)
                    nc.sync.dma_start(xc_sb[:, ct, s], xc_d[:, ct, s])

            # ---- Q projection: q_sb[d, q] (1/16 folded into host weights) ----
            for qc in range(QH // 512):
                s = slice(qc * 512, (qc + 1) * 512)
                for dt in range(CT):
                    ps = mmp.tile([128, 512], f32, tag="mm")
                    for ct in range(CT):
                        nc.tensor.matmul(
                            ps[:], wq_sb[:, ct, dt * 128:(dt + 1) * 128],
                            xq_sb[:, ct, s],
                            start=(ct == 0), stop=(ct == CT - 1))
                    nc.scalar.activation(q_sb[:, dt, s], ps[:],
                                         AF.Identity, bias=bq_sb[:, dt:dt + 1])

            # ---- K projection: k_sb[d, k] ----
            for kc in range(KC):
                s = slice(kc * 512, (kc + 1) * 512)
                for dt in range(CT):
                    ps = mmp.tile([128, 512], f32, tag="mm")
                    for ct in range(CT):
                        nc.tensor.matmul(
                            ps[:], wk_sb[:, ct, dt * 128:(dt + 1) * 128],
                            xc_sb[:, ct, s],
                            start=(ct == 0), stop=(ct == CT - 1))
                    nc.scalar.activation(k_sb[:, dt, s], ps[:],
                                         AF.Identity, bias=bk_sb[:, dt:dt + 1])

            # ---- U^T = x_cnn^T Wu^T  [keys, 256] (no bias; it's constant
            # after softmax and folded into bf') ----
            for mt in range(NKT):
                ps = mmp.tile([128, 512], f32, tag="mm")
                for ct in range(CT):
                    nc.tensor.matmul(
                        ps[:, :C], xc_sb[:, ct, mt * 128:(mt + 1) * 128],
                        wu_sb[:, ct],
                        start=(ct == 0), stop=(ct == CT - 1))
                nc.vector.tensor_copy(ut_sb[:, mt, :C], ps[:, :C])

            # ---- attention + fused conv, per 512-query superblock ----
            for sb in range(NSB):
                qs = slice(sb * 512, (sb + 1) * 512)
                pt_sb = ptp.tile([128, NKT, 512], f16, tag="pt")
                # S^T = K_kt^T Q_sb ; P^T = exp(S^T)
                for kt in range(NKT):
                    ps = mmp.tile([128, 512], f32, tag="mm")
                    for ct in range(CT):
                        nc.tensor.matmul(
                            ps[:], k_sb[:, ct, kt * 128:(kt + 1) * 128],
                            q_sb[:, ct, qs],
                            start=(ct == 0), stop=(ct == CT - 1))
                    nc.scalar.activation(pt_sb[:, kt], ps[:], AF.Exp)

                # conv part 1 into psO (queries free, channels partitions)
                pso = [pop.tile([128, 512], f32, tag=f"po{et}", name=f"pso{et}")
                       for et in range(CT)]
                for et in range(CT):
                    for ct in range(CT):
                        nc.tensor.matmul(
                            pso[et][:], wf_sb[:, ct, et * 128:(et + 1) * 128],
                            xq_sb[:, ct, qs],
                            start=(ct == 0), stop=False)

                # PV: [A | R] per 128-query block, then normalize on DVE
                c_blk = []
                for qj in range(4):
                    psb = pvp.tile([128, C + 1], f32, tag="pv")
                    for kt in range(NKT):
                        nc.tensor.matmul(
                            psb[:], pt_sb[:, kt, qj * 128:(qj + 1) * 128],
                            ut_sb[:, kt],
                            start=(kt == 0), stop=(kt == NKT - 1))
                    rinv = cbp.tile([128, 1], f32, tag="rinv")
                    nc.vector.reciprocal(rinv[:], psb[:, C:C + 1])
                    c_sb = cbp.tile([128, C], f16, tag="c")
                    nc.vector.tensor_scalar_mul(c_sb[:], psb[:, :C], rinv[:])
                    c_blk.append(c_sb)

                # transpose each c block into the psO accumulation via
                # identity-matmul; last one closes the group
                for qj in range(4):
                    for et in range(CT):
                        nc.tensor.matmul(
                            pso[et][:, qj * 128:(qj + 1) * 128],
                            c_blk[qj][:, et * 128:(et + 1) * 128],
                            ident[:],
                            start=False, stop=(qj == 3),
                            skip_group_check=True)

                for et in range(CT):
                    o_sb = outp.tile([128, 512], f32, tag="o")
                    nc.scalar.activation(o_sb[:], pso[et][:],
                                         AF.Identity, bias=bf_sb[:, et:et + 1])
                    nc.sync.dma_start(out_d[:, et, qs], o_sb[:])
    nc.finalize()
    return nc


def _get_nc():
    if "nc" not in _CACHE:
        _CACHE["nc"] = _build()
    return _CACHE["nc"]


def _in_maps(transformer_features, cnn_features, Wq, bq, Wk, bk, Wv, bv, Wf, bf):
    xt = np.ascontiguousarray(np.asarray(transformer_features, np.float32)
                              .reshape(B, C, N))
    xc = np.ascontiguousarray(np.asarray(cnn_features, np.float32)
                              .reshape(B, C, N))
    Wq = np.asarray(Wq, np.float32)
    Wk = np.asarray(Wk, np.float32)
    Wv = np.asarray(Wv, np.float32)
    Wf = np.asarray(Wf, np.float32)
    bq = np.asarray(bq, np.float32)
    bk = np.asarray(bk, np.float32)
    bv = np.asarray(bv, np.float32)
    bf = np.asarray(bf, np.float32)

    Wf1, Wf2 = Wf[:, :C], Wf[:, C:]
    wqt = np.ascontiguousarray(Wq.T / 16.0)
    wkt = np.ascontiguousarray(Wk.T)
    wut = np.ascontiguousarray((Wf2 @ Wv).T)
    wf1 = np.ascontiguousarray(Wf1.T)
    bq_s = bq / 16.0
    bf2 = bf + Wf2 @ bv

    maps = []
    for c in range(NCORES):
        b, h = divmod(c, 2)
        maps.append(dict(
            xq=np.ascontiguousarray(xt[b][:, h * QH:(h + 1) * QH]),
            xc=xc[b],
            wqt=wqt, wkt=wkt, wut=wut, wf1=wf1,
            bq=bq_s, bk=bk, bf=bf2,
        ))
    return maps


def _run(inputs, trace=False):
    from concourse.bass_utils import run_bass_kernel_spmd
    nc = _get_nc()
    maps = _in_maps(**inputs)
    return run_bass_kernel_spmd(nc, maps, list(range(NCORES)), trace=trace)


def kernel(**inputs) -> np.ndarray:
    res = _run(inputs).results
    out = np.empty((B, C, N), np.float32)
    for c in range(NCORES):
        b, h = divmod(c, 2)
        out[b][:, h * QH:(h + 1) * QH] = res[c]["out"]
    return out.reshape(B, C, H, W)


# revision 5
# speedup vs baseline: 1.6455x; 1.2194x over previous
"""CrossAttentionFusion Trainium2 kernel (nn_CrossAttentionFusion__45561013076033).

Full inputs -> full output. Sharding: 8 cores, core c handles batch b=c//2,
query-half h=c%2 (2048 of 4096 queries). Each core holds the full [256,4096]
cnn feature map of its batch (keys), its query-half of the transformer
features, and replicated weights.

Key restructurings vs the naive dataflow:
  * out = Wf1 @ x_trf + Wf2 @ attended + bf'.  Fold Wf2 into the value
    projection: U = (Wf2 @ Wv) @ x_cnn, so attention directly produces
    conv-ready channels; bv's contribution is constant (softmax rows sum
    to 1) and lands in bf' = bf + Wf2 @ bv.
  * Scores are computed pre-transposed, S^T[k, q] = K_kt^T Q, as ONE fp8
    DoubleRow matmul per 128-key tile (256-deep contraction in a single
    pass; Q/K are written in fp8e4m3 straight out of the projections; the
    1/sqrt(dim) scale is applied by the exp activation's free affine).
  * Softmax row-sums come free as a 257th column of the PV matmul by
    augmenting U^T with a constant column: [A | 16R] = P [16U | 16].
    (16x scale keeps (Wf2 Wv) entries well inside fp8/fp16 range; it
    cancels exactly in A * (1/16R).)
  * Normalization is a per-partition scalar multiply on the vector engine
    while moving the PV result PSUM->SBUF (queries sit on partitions).
  * The [q, e] -> [e, q] layout fix-up is a matmul with a 128x128 identity
    as the moving operand, accumulated directly into the Wf1 PSUM group.
"""

import numpy as np

B, C, H, W = 4, 256, 64, 64
N = H * W            # 4096 tokens
NCORES = 8
QH = N // 2          # 2048 queries per core
CT = C // 128        # 2 channel tiles
KC = N // 512        # 8 key chunks of 512
NSB = QH // 512      # 4 superblocks per core
NKT = N // 128       # 32 key tiles

_CACHE = {}


def _build():
    import concourse.bass as bass
    import concourse.mybir as mybir
    import concourse.tile as tile
    from concourse import bacc
    from concourse.masks import make_identity

    f32 = mybir.dt.float32
    f32r = mybir.dt.float32r
    f16 = mybir.dt.float16
    f8 = mybir.dt.float8e4
    AF = mybir.ActivationFunctionType
    DR = mybir.MatmulPerfMode.DoubleRow

    nc = bacc.Bacc("TRN2", target_bir_lowering=False, debug=True)

    XQ = nc.dram_tensor("xq", [C, QH], f32, kind="ExternalInput")
    XC = nc.dram_tensor("xc", [C, N], f32, kind="ExternalInput")
    WQT = nc.dram_tensor("wqt", [C, C], f32, kind="ExternalInput")
    WKT = nc.dram_tensor("wkt", [C, C], f32, kind="ExternalInput")
    WUT = nc.dram_tensor("wut", [C, C], f32, kind="ExternalInput")
    WF1 = nc.dram_tensor("wf1", [C, C], f32, kind="ExternalInput")
    BQ = nc.dram_tensor("bq", [C], f32, kind="ExternalInput")
    BK = nc.dram_tensor("bk", [C], f32, kind="ExternalInput")
    BF = nc.dram_tensor("bf", [C], f32, kind="ExternalInput")
    OUT = nc.dram_tensor("out", [C, QH], f32, kind="ExternalOutput")

    xq_d = XQ.ap().bitcast(f32r).rearrange("(t p) n -> p t n", p=128)
    xc_d = XC.ap().bitcast(f32r).rearrange("(t p) n -> p t n", p=128)
    wq_d = WQT.ap().bitcast(f32r).rearrange("(t p) d -> p t d", p=128)
    wk_d = WKT.ap().bitcast(f32r).rearrange("(t p) d -> p t d", p=128)
    wu_d = WUT.ap().bitcast(f32r).rearrange("(t p) d -> p t d", p=128)
    wf_d = WF1.ap().bitcast(f32r).rearrange("(t p) d -> p t d", p=128)
    out_d = OUT.ap().rearrange("(t p) n -> p t n", p=128)

    with tile.TileContext(nc) as tc:
        with tc.tile_pool(name="persist", bufs=1) as per, \
             tc.tile_pool(name="pt", bufs=2) as ptp, \
             tc.tile_pool(name="cb", bufs=4) as cbp, \
             tc.tile_pool(name="outp", bufs=2) as outp, \
             tc.tile_pool(name="mm", bufs=2, space="PSUM") as mmp, \
             tc.tile_pool(name="pv", bufs=2, space="PSUM") as pvp, \
             tc.tile_pool(name="po", bufs=1, space="PSUM") as pop:

            # ---- persistent tiles ----
            xq_sb = per.tile([128, CT, QH], f32r)
            xc_sb = per.tile([128, CT, N], f32r)
            wq_sb = per.tile([128, CT, C], f32r)
            wk_sb = per.tile([128, CT, C], f32r)
            wu_sb = per.tile([128, CT, C], f32r)
            wf_sb = per.tile([128, CT, C], f32r)
            bq_sb = per.tile([128, CT], f32)
            bk_sb = per.tile([128, CT], f32)
            bf_sb = per.tile([128, CT], f32)
            q8_sb = per.tile([128, CT, QH], f8)
            k8_sb = per.tile([128, CT, N], f8)
            xc8_sb = per.tile([128, CT, N], f8)
            wu8_sb = per.tile([128, CT, C], f8)
            ut_sb = per.tile([128, NKT, C + 1], f16)
            ident = per.tile([128, 128], f16)

            nc.sync.dma_start(bq_sb[:], BQ.ap().rearrange("(t p) -> p t", p=128))
            nc.sync.dma_start(bk_sb[:], BK.ap().rearrange("(t p) -> p t", p=128))
            nc.sync.dma_start(bf_sb[:], BF.ap().rearrange("(t p) -> p t", p=128))
            nc.sync.dma_start(wq_sb[:], wq_d)
            make_identity(nc, ident[:])
            nc.gpsimd.memset(ut_sb[:, :, C:C + 1], 16.0)

            # input DMAs: xq/xc interleaved 512-column chunks so both Q and
            # K projections start early; remaining weights between chunks
            for i in range(KC):
                if i < QH // 512:
                    for ct in range(CT):
                        s = slice(i * 512, (i + 1) * 512)
                        nc.sync.dma_start(xq_sb[:, ct, s], xq_d[:, ct, s])
                if i == 0:
                    nc.sync.dma_start(wk_sb[:], wk_d)
                elif i == 1:
                    nc.sync.dma_start(wu_sb[:], wu_d)
                elif i == 2:
                    nc.sync.dma_start(wf_sb[:], wf_d)
                for ct in range(CT):
                    s = slice(i * 512, (i + 1) * 512)
                    nc.sync.dma_start(xc_sb[:, ct, s], xc_d[:, ct, s])

            # fp8 side copies for the DoubleRow matmuls
            nc.vector.tensor_scalar_mul(wu8_sb[:], wu_sb[:], 16.0)

            # ---- Q projection -> fp8 (scores keep full scale; the 1/16
            # softmax scale is applied inside the exp activation) ----
            for qc in range(QH // 512):
                s = slice(qc * 512, (qc + 1) * 512)
                for dt in range(CT):
                    ps = mmp.tile([128, 2, 512], f32, tag="mm")
                    for ct in range(CT):
                        nc.tensor.matmul(
                            ps[:, 0], wq_sb[:, ct, dt * 128:(dt + 1) * 128],
                            xq_sb[:, ct, s],
                            start=(ct == 0), stop=(ct == CT - 1))
                    nc.scalar.activation(q8_sb[:, dt, s], ps[:, 0],
                                         AF.Identity, bias=bq_sb[:, dt:dt + 1])

            # ---- K projection -> fp8 ----
            for kc in range(KC):
                s = slice(kc * 512, (kc + 1) * 512)
                for dt in range(CT):
                    ps = mmp.tile([128, 2, 512], f32, tag="mm")
                    for ct in range(CT):
                        nc.tensor.matmul(
                            ps[:, 0], wk_sb[:, ct, dt * 128:(dt + 1) * 128],
                            xc_sb[:, ct, s],
                            start=(ct == 0), stop=(ct == CT - 1))
                    nc.scalar.activation(k8_sb[:, dt, s], ps[:, 0],
                                         AF.Identity, bias=bk_sb[:, dt:dt + 1])
                nc.vector.tensor_copy(xc8_sb[:, :, s], xc_sb[:, :, s])

            # ---- U^T = x_cnn^T (16 Wu)^T  [keys, 256], fp8 DoubleRow ----
            for g in range(NKT // 2):
                ps = mmp.tile([128, 2, 512], f32, tag="mm")
                for j in range(2):
                    mt = 2 * g + j
                    nc.tensor.matmul(
                        ps[:, j, :C], xc8_sb[:, :, mt * 128:(mt + 1) * 128],
                        wu8_sb[:], perf_mode=DR, start=True, stop=True)
                nc.vector.tensor_copy(ut_sb[:, 2 * g:2 * g + 2, :C],
                                      ps[:, :, :C])

            # ---- attention + fused conv, per 512-query superblock ----
            for sb in range(NSB):
                qs = slice(sb * 512, (sb + 1) * 512)
                pt_sb = ptp.tile([128, NKT, 512], f16, tag="pt")
                # S^T = K_kt^T Q_sb (fp8 DoubleRow, full 256-contraction in
                # one matmul); P^T = exp(S^T / 16), two key tiles per act
                for g in range(NKT // 2):
                    ps = mmp.tile([128, 2, 512], f32, tag="mm")
                    for j in range(2):
                        kt = 2 * g + j
                        nc.tensor.matmul(
                            ps[:, j], k8_sb[:, :, kt * 128:(kt + 1) * 128],
                            q8_sb[:, :, qs], perf_mode=DR,
                            start=True, stop=True)
                    nc.scalar.activation(pt_sb[:, 2 * g:2 * g + 2], ps[:],
                                         AF.Exp, scale=0.0625)

                # conv part 1 into psO (queries free, channels partitions)
                pso = [pop.tile([128, 512], f32, tag=f"po{et}", name=f"pso{et}")
                       for et in range(CT)]
                for et in range(CT):
                    for ct in range(CT):
                        nc.tensor.matmul(
                            pso[et][:], wf_sb[:, ct, et * 128:(et + 1) * 128],
                            xq_sb[:, ct, qs],
                            start=(ct == 0), stop=False)

                # PV: [A | 16R] per 128-query block, then normalize on DVE
                c_blk = []
                for qj in range(4):
                    psb = pvp.tile([128, C + 1], f32, tag="pv")
                    for kt in range(NKT):
                        nc.tensor.matmul(
                            psb[:], pt_sb[:, kt, qj * 128:(qj + 1) * 128],
                            ut_sb[:, kt],
                            start=(kt == 0), stop=(kt == NKT - 1))
                    rinv = cbp.tile([128, 1], f32, tag="rinv")
                    nc.vector.reciprocal(rinv[:], psb[:, C:C + 1])
                    c_sb = cbp.tile([128, C], f16, tag="c")
                    nc.vector.tensor_scalar_mul(c_sb[:], psb[:, :C], rinv[:])
                    c_blk.append(c_sb)

                # transpose each c block into the psO accumulation via
                # identity-matmul; last one closes the group
                for qj in range(4):
                    for et in range(CT):
                        nc.tensor.matmul(
                            pso[et][:, qj * 128:(qj + 1) * 128],
                            c_blk[qj][:, et * 128:(et + 1) * 128],
                            ident[:],
                            start=False, stop=(qj == 3),
                            skip_group_check=True)

                for et in range(CT):
                    o_sb = outp.tile([128, 512], f32, tag="o")
                    nc.scalar.activation(o_sb[:], pso[et][:],
                                         AF.Identity, bias=bf_sb[:, et:et + 1])
                    nc.sync.dma_start(out_d[:, et, qs], o_sb[:])
    nc.finalize()
    return nc


def _get_nc():
    if "nc" not in _CACHE:
        _CACHE["nc"] = _build()
    return _CACHE["nc"]


def _in_maps(transformer_features, cnn_features, Wq, bq, Wk, bk, Wv, bv, Wf, bf):
    xt = np.ascontiguousarray(np.asarray(transformer_features, np.float32)
                              .reshape(B, C, N))
    xc = np.ascontiguousarray(np.asarray(cnn_features, np.float32)
                              .reshape(B, C, N))
    Wq = np.asarray(Wq, np.float32)
    Wk = np.asarray(Wk, np.float32)
    Wv = np.asarray(Wv, np.float32)
    Wf = np.asarray(Wf, np.float32)
    bq = np.asarray(bq, np.float32)
    bk = np.asarray(bk, np.float32)
    bv = np.asarray(bv, np.float32)
    bf = np.asarray(bf, np.float32)

    Wf1, Wf2 = Wf[:, :C], Wf[:, C:]
    wqt = np.ascontiguousarray(Wq.T)
    wkt = np.ascontiguousarray(Wk.T)
    wut = np.ascontiguousarray((Wf2 @ Wv).T)
    wf1 = np.ascontiguousarray(Wf1.T)
    bf2 = bf + Wf2 @ bv

    maps = []
    for c in range(NCORES):
        b, h = divmod(c, 2)
        maps.append(dict(
            xq=np.ascontiguousarray(xt[b][:, h * QH:(h + 1) * QH]),
            xc=xc[b],
            wqt=wqt, wkt=wkt, wut=wut, wf1=wf1,
            bq=bq, bk=bk, bf=bf2,
        ))
    return maps


def _run(inputs, trace=False):
    from concourse.bass_utils import run_bass_kernel_spmd
    nc = _get_nc()
    maps = _in_maps(**inputs)
    return run_bass_kernel_spmd(nc, maps, list(range(NCORES)), trace=trace)


def kernel(**inputs) -> np.ndarray:
    res = _run(inputs).results
    out = np.empty((B, C, N), np.float32)
    for c in range(NCORES):
        b, h = divmod(c, 2)
        out[b][:, h * QH:(h + 1) * QH] = res[c]["out"]
    return out.reshape(B, C, H, W)
